# revision 1
# baseline (speedup 1.0000x reference)
"""Trainium2 Bass kernel for nn_DeformKernelConv2d.

Math (per batch image; shapes below are per core after sharding):
  offsets:  off = conv3x3(x, offset_w) + offset_b          -> dy,dx per (k, pixel)
  coords:   yc_k = dy_k + by_k ; xc_k = dx_k + bx_k        (scope-kernel space)
  phi:      phi_y[k,i] = relu(1-|yc_k - i|), i=0..3        (likewise phi_x)
  Phi:      Phi_k[4*yi+xi] = phi_y[k,yi] * phi_x[k,xi]     (bilinear weights, 16 per k)
  samp:     samp_k[c] = sum_s wflat[c,s] * Phi_k[s]        (matmul over s=16)
  out:      out[c] = sum_k samp_k[c] * x_k[c]              (x_k = 3x3-shifted x)

Device mapping (v5):
  - 8 cores: (batch b, H-half); each core does 28 rows with a 1-row halo.
  - offset conv fused with the affine expansion into a 9-tap accumulated
    matmul -> T[72, pix]; u = |T + bias| in one ACT op.
  - row replication of u into 128-row (k,s) patterns via PE matmuls against
    0/1 selection matrices; k=8 rides in rows 16..31 of the A tile and its
    samp matmul contracts rows 0..31 against a half-zero [32,128] weight.
  - phi = relu(1 - u) fused into the PSUM->SBUF drains (ACT).
  - chunk rows are uneven (9/9/8/2): big chunks amortize per-chunk matmul
    overhead and give the PE long dense bursts; the last chunk is tiny so
    the end-exposed tail is short.
  - software pipelined: chunk ch's taps are emitted before chunk ch-1's
    phi/samp stages.
  - samp matmuls pair-packed into 2-bank PSUM tiles, drained 2 k-planes per
    ACT instruction; tail (prod mul + 9-way k-sum tree) entirely on DVE.
"""

import numpy as np
import ml_dtypes

B, C, H, W = 4, 128, 56, 56
HC = H // 2            # 28 rows per core
NPIX = HC * W          # 1568
RA, RB = 58, 60        # padded row lengths: xbfA data at col 1, xbfB at col 2

CH_ROWS = [9, 9, 8, 2]
CH_R0 = [0, 9, 18, 26]
NCH = 4
# tails: (emit-after-chunk, pixel range, row range)
TAILS = [(1, 0, 18), (2, 18, 26), (3, 26, 28)]

_BF16 = ml_dtypes.bfloat16
_cache = {}


def _build_program():
    import concourse.tile as tile
    import concourse.mybir as mybir
    from concourse import bacc

    fp32 = mybir.dt.float32
    bf16 = mybir.dt.bfloat16
    AF = mybir.ActivationFunctionType

    nc = bacc.Bacc("TRN2", target_bir_lowering=False, debug=False, num_devices=8)
    lhsT_d = nc.dram_tensor("lhsT", [C, 648], bf16, kind="ExternalInput")
    # consts2 cols: [0:128] w4T | [128:256] w8T(rows<32) | [256:512] SA | [512:768] SB
    consts2_d = nc.dram_tensor("consts2", [C, 768], bf16, kind="ExternalInput")
    bias_d = nc.dram_tensor("bias72", [72, 1], fp32, kind="ExternalInput")
    xsA_d = nc.dram_tensor("xsA", [C, HC + 2, RA], bf16, kind="ExternalInput")
    xsB_d = nc.dram_tensor("xsB", [C, HC + 2, RB], bf16, kind="ExternalInput")
    out_d = nc.dram_tensor("out", [C, HC, W], fp32, kind="ExternalOutput")

    with tile.TileContext(nc) as tc:
        with (
            tc.tile_pool(name="const", bufs=1) as cp,
            tc.tile_pool(name="work", bufs=1) as wp,
            tc.tile_pool(name="tmp", bufs=2) as tp,
            tc.tile_pool(name="psT", bufs=2, space="PSUM") as ppT,
            tc.tile_pool(name="rep", bufs=1, space="PSUM") as ppR,
            tc.tile_pool(name="psS", bufs=2, space="PSUM") as ppS,
        ):
            lhsT = cp.tile([C, 648], bf16)
            consts2 = cp.tile([C, 768], bf16)
            bias = cp.tile([72, 1], fp32)
            xbfA = cp.tile([C, HC + 2, RA], bf16)
            xbfB = cp.tile([C, HC + 2, RB], bf16)

            w4T = consts2[:, 0:128]
            w8T = consts2[0:32, 128:256]
            SA = consts2[0:72, 256:512]
            SB = consts2[0:72, 512:768]

            # loads split across the two HWDGE queues; one DMA per tile so
            # consumers gate on exactly the data they need
            nc.sync.dma_start(xbfA[:], xsA_d[:])
            nc.sync.dma_start(bias[:], bias_d[:])
            nc.scalar.dma_start(lhsT[:], lhsT_d[:])
            nc.scalar.dma_start(consts2[:], consts2_d[:])
            nc.scalar.dma_start(xbfB[:], xsB_d[:])

            samp = wp.tile([C, 9, NPIX], bf16)
            prod = wp.tile([C, 9, NPIX], bf16)

            u_tiles = {}
            psT_tiles = {}

            def emit_taps_pair(chs):
                # tap-major over a pair of chunks: one LDWEIGHTS per tap
                # feeds both chunks' matmuls back-to-back
                for ch in chs:
                    psT = ppT.tile([72, 512], fp32, tag="psT")
                    psT_tiles[ch] = psT
                for tap in range(9):
                    di, dj = tap // 3, tap % 3
                    for ch in chs:
                        R = CH_ROWS[ch]
                        r0 = CH_R0[ch]
                        N = R * W
                        rhs = xbfA[:, r0 + di : r0 + di + R, dj : dj + W]
                        nc.tensor.matmul(
                            psT_tiles[ch][:, 0:N],
                            lhsT[:, tap * 72 : (tap + 1) * 72],
                            rhs,
                            start=(tap == 0),
                            stop=(tap == 8),
                            skip_group_check=True,
                        )
                for ch in chs:
                    N = CH_ROWS[ch] * W
                    u = tp.tile([72, 512], bf16, tag="u")
                    u_tiles[ch] = u
                    nc.scalar.activation(
                        u[:, 0:N], psT_tiles[ch][:, 0:N], AF.Abs,
                        bias=bias[:], scale=1.0,
                    )

            def emit_phi_samp(ch):
                R = CH_ROWS[ch]
                N = R * W
                c0 = CH_R0[ch] * W
                u = u_tiles.pop(ch)
                YXA = tp.tile([C, 2, 512], bf16, tag="yxa")
                YXB = tp.tile([C, 2, 512], bf16, tag="yxb")
                PhA = tp.tile([C, 512], bf16, tag="phA")
                PhB = tp.tile([C, 512], bf16, tag="phB")
                UA = ppR.tile([C, 2, 512], fp32, tag="rep")
                nc.tensor.matmul(UA[:, 0, 0:N], SA[:, 0:128], u[:, 0:N], start=True, stop=True)
                nc.tensor.matmul(UA[:, 1, 0:N], SA[:, 128:256], u[:, 0:N], start=True, stop=True)
                nc.scalar.activation(YXA[:, :, 0:N], UA[:, :, 0:N], AF.Relu, bias=1.0, scale=-1.0)
                UB = ppR.tile([C, 2, 512], fp32, tag="rep")
                nc.tensor.matmul(UB[:, 0, 0:N], SB[:, 0:128], u[:, 0:N], start=True, stop=True)
                nc.tensor.matmul(UB[:, 1, 0:N], SB[:, 128:256], u[:, 0:N], start=True, stop=True)
                nc.scalar.activation(YXB[:, :, 0:N], UB[:, :, 0:N], AF.Relu, bias=1.0, scale=-1.0)

                nc.vector.tensor_mul(PhA[:, 0:N], YXA[:, 0, 0:N], YXA[:, 1, 0:N])
                nc.vector.tensor_mul(PhB[:, 0:N], YXB[:, 0, 0:N], YXB[:, 1, 0:N])

                def samp_mm(k, dst_ps):
                    if k == 8:
                        nc.tensor.matmul(
                            dst_ps, w8T, PhA[0:32, 0:N],
                            start=True, stop=True, tile_position=(0, 0),
                        )
                    else:
                        g = k % 4
                        src = PhA if k < 4 else PhB
                        base = 32 * g
                        nc.tensor.matmul(
                            dst_ps,
                            w4T[base : base + 16, :],
                            src[base : base + 16, 0:N],
                            start=True, stop=True, tile_position=(base, 0),
                        )

                csl = slice(c0, c0 + N)
                for p in range(4):
                    psS = ppS.tile([C, 2, 512], fp32, tag="psS")
                    samp_mm(2 * p, psS[:, 0, 0:N])
                    samp_mm(2 * p + 1, psS[:, 1, 0:N])
                    nc.scalar.copy(samp[:, 2 * p : 2 * p + 2, csl], psS[:, :, 0:N])
                psS = ppS.tile([C, 2, 512], fp32, tag="psS")
                samp_mm(8, psS[:, 0, 0:N])
                nc.scalar.copy(samp[:, 8, csl], psS[:, 0, 0:N])

            def emit_tail(row0, row1):
                c0, c1 = row0 * W, row1 * W
                HH = row1 - row0
                pv = prod[:, :, c0:c1]
                for k in range(9):
                    di, dj = k // 3, k % 3
                    if dj == 1:
                        xsrc, coff = xbfB, 2
                    else:
                        xsrc, coff = xbfA, dj
                    xv = xsrc[:, row0 + di : row0 + di + HH, coff : coff + W]
                    nc.vector.tensor_mul(
                        prod[:, k, c0:c1].rearrange("p (h w) -> p h w", h=HH),
                        samp[:, k, c0:c1].rearrange("p (h w) -> p h w", h=HH),
                        xv,
                    )
                t1a = tp.tile([C, 2, 1008], bf16, tag="t1a")
                t1b = tp.tile([C, 2, 1008], bf16, tag="t1b")
                t2 = tp.tile([C, 2, 1008], bf16, tag="t2")
                t3 = tp.tile([C, 1008], bf16, tag="t3")
                res = tp.tile([C, 1008], fp32, tag="res")
                N = c1 - c0
                nc.vector.tensor_add(t1a[:, :, 0:N], pv[:, 0:2, :], pv[:, 2:4, :])
                nc.vector.tensor_add(t1b[:, :, 0:N], pv[:, 4:6, :], pv[:, 6:8, :])
                nc.vector.tensor_add(t2[:, :, 0:N], t1a[:, :, 0:N], t1b[:, :, 0:N])
                nc.vector.tensor_add(t3[:, 0:N], t2[:, 0, 0:N], t2[:, 1, 0:N])
                nc.vector.tensor_add(res[:, 0:N], t3[:, 0:N], pv[:, 8, :])
                nc.sync.dma_start(
                    out_d[:, row0:row1, :],
                    res[:, 0:N].rearrange("p (h w) -> p h w", h=HH),
                )

            # pipeline: taps for a chunk pair, then phi/samp per chunk with
            # the next pair's taps issued in between
            emit_taps_pair([0, 1])
            emit_phi_samp(0)
            emit_taps_pair([2, 3])
            emit_phi_samp(1)
            emit_tail(0, 18)
            emit_phi_samp(2)
            emit_tail(18, 26)
            emit_phi_samp(3)
            emit_tail(26, 28)

    nc.finalize()
    return nc


def _prep_inputs(x, offset_w, offset_b, weight):
    """Host-side sharding + weight reshaping. Returns per-core input maps."""
    x = np.asarray(x, dtype=np.float32)
    offset_w = np.asarray(offset_w, dtype=np.float32)
    offset_b = np.asarray(offset_b, dtype=np.float32)
    weight = np.asarray(weight, dtype=np.float32)

    # lhsT[c, tap*72 + k*8 + axis*4 + i] = offset_w[2k+axis, c, tap//3, tap%3]
    ow = offset_w.reshape(9, 2, C, 3, 3)  # [k, axis, c, di, dj]
    lhsT = np.transpose(ow, (2, 3, 4, 0, 1))  # [c, di, dj, k, axis]
    lhsT = np.repeat(lhsT[..., None], 4, axis=-1)  # [c, di, dj, k, axis, i]
    lhsT = np.ascontiguousarray(lhsT.reshape(C, 648)).astype(_BF16)

    # w4T rows 32g+s = weight[:, s//4, s%4]; w8T rows 16..31 = same
    wT = weight.reshape(C, 16).T  # [16, C]
    w4T = np.zeros((C, 128), dtype=np.float32)
    for g in range(4):
        w4T[32 * g : 32 * g + 16, :] = wT
    w8T = np.zeros((C, 128), dtype=np.float32)
    w8T[16:32, :] = wT

    # selection matrices [72, 128]: row r = k*8 + axis*4 + i
    SAy = np.zeros((C, 128), dtype=np.float32)
    SAx = np.zeros((C, 128), dtype=np.float32)
    SBy = np.zeros((C, 128), dtype=np.float32)
    SBx = np.zeros((C, 128), dtype=np.float32)
    for k in range(4):
        for s in range(16):
            yi, xi = s // 4, s % 4
            SAy[k * 8 + yi, 32 * k + s] = 1.0
            SAx[k * 8 + 4 + xi, 32 * k + s] = 1.0
            SBy[(k + 4) * 8 + yi, 32 * k + s] = 1.0
            SBx[(k + 4) * 8 + 4 + xi, 32 * k + s] = 1.0
    # k=8 patterns ride in A rows 16..31
    for s in range(16):
        yi, xi = s // 4, s % 4
        SAy[64 + yi, 16 + s] = 1.0
        SAx[64 + 4 + xi, 16 + s] = 1.0

    consts2 = np.concatenate(
        [w4T, w8T, np.concatenate([SAy, SAx], axis=1),
         np.concatenate([SBy, SBx], axis=1)], axis=1
    ).astype(_BF16)

    # bias72[k*8+axis*4+i] = offset_b[2k+axis] + base - i
    base = np.arange(3, dtype=np.float32) + 0.5
    bias = np.zeros((9, 2, 4), dtype=np.float32)
    for k in range(9):
        for axis in range(2):
            bv = base[k // 3] if axis == 0 else base[k % 3]
            bias[k, axis, :] = offset_b[2 * k + axis] + bv - np.arange(4)
    bias72 = bias.reshape(72, 1)

    xb = x.astype(_BF16)
    in_maps = []
    for core in range(8):
        b, half = core // 2, core % 2
        h0 = half * HC
        xsA = np.zeros((C, HC + 2, RA), dtype=_BF16)
        xsB = np.zeros((C, HC + 2, RB), dtype=_BF16)
        lo, hi = h0 - 1, h0 + HC + 1
        slo, shi = max(lo, 0), min(hi, H)
        xsA[:, slo - lo : slo - lo + (shi - slo), 1:57] = xb[b, :, slo:shi, :]
        xsB[:, slo - lo : slo - lo + (shi - slo), 2:58] = xb[b, :, slo:shi, :]
        in_maps.append(
            {"lhsT": lhsT, "consts2": consts2, "bias72": bias72,
             "xsA": xsA, "xsB": xsB}
        )
    return in_maps


def kernel(x, offset_w, offset_b, weight):
    from concourse.bass_utils import run_bass_kernel_spmd

    if "nc" not in _cache:
        _cache["nc"] = _build_program()
    nc = _cache["nc"]

    in_maps = _prep_inputs(x, offset_w, offset_b, weight)
    res = run_bass_kernel_spmd(nc, in_maps, core_ids=list(range(8)))

    out = np.zeros((B, C, H, W), dtype=np.float32)
    for core in range(8):
        b, half = core // 2, core % 2
        out[b, :, half * HC : (half + 1) * HC, :] = res.results[core]["out"].reshape(
            C, HC, W
        )
    return out

